# Initial kernel scaffold
#
"""Trainium2 Bass kernel for nn_Capsule (capsule routing with dynamic routing).

reference: u = x @ W  (per-sample [512,256]@[256,512] -> [512, (32 o, 16 f)])
           b=0; 3x { c = softmax_o(b); v[o,f] = sum_i c[o,i] u[i,(o,f)];
                     v = squash(v); b[o,i] = sum_f v[o,f] u[i,(o,f)] }
           return v  [B, 32, 16]

Key algebraic restructuring (u is NEVER materialized):
  v_raw[o,f] = sum_i c[o,i] u[i,(o,f)]  =  diag-extract[ (c @ x) @ W ]
      y = c @ x      (PE: cT stationary [i,32], x natural moving)
      vfull = y @ W  (PE: yT stationary, W natural moving)
      v_raw = mask * vfull, then per-sample partition-sum via indicator matmul
  b[o,i] = sum_f v[o,f] u[i,(o,f)] = sum_h z[o,h] x[i,h]
      z[o,h] = sum_f v[o,f] W[h,(o,f)]  (PE: block-diag Vmat stationary, WT moving)
      b = z @ xT     (PE: zT stationary col-tiled, xT moving)
  softmax over o on bT [i-partition, o-free] via PE transposes of exp(b).

16 samples/core x 8 cores; per core 2 half-batches of 2 packs x 4 samples;
a pack's 4 samples run concurrently via PE column tiling tile_position=(0,32s).
"""

import numpy as np

import concourse.bass as bass
import concourse.tile as tile
from concourse import mybir
from concourse.bass_utils import run_bass_kernel_spmd

F32 = mybir.dt.float32
R32 = mybir.dt.float32r
AF = mybir.ActivationFunctionType
AX = mybir.AxisListType

B, I, H = 128, 512, 256
O, F = 32, 16
OF = O * F  # 512
NCORES = 8
S = B // NCORES      # 16 samples per core
NHB = 2              # half-batches per core
NPK = 2              # packs per half-batch
PK = 4               # samples per pack (col-tiling width)
NITER = 3
P = 128

# constant-blob layout (one DMA, per-partition element offsets)
CW = 0                  # W  [h%128, (hc 2, of 512)]
CWT = CW + 2 * OF       # WT [of%128, (m 4, h 256)]
CID = CWT + 4 * H       # identity [128, 128]
CMC = CID + P           # diag mask [128, 512]
CS4 = CMC + OF          # sample-sum indicator [128, 4]
CBM = CS4 + PK          # Vmat block masks [128, (m 4, j 32)]
CC0 = CBM + 4 * O       # uniform 1/32 [128, 32]
CIDR = CC0 + O          # identity again, viewed as float32r by device
CSTN = CIDR + P


def ap(t, dims, off=0):
    """AP over tile/handle `t`: keep partition dim, explicit free dims."""
    a = t if isinstance(t, bass.AP) else t[:]
    return bass.AP(tensor=a.tensor, offset=a.offset + off,
                   ap=[list(a.ap[0])] + [list(d) for d in dims])


def fview(a):
    """Alias a float32r AP as plain fp32 (same bytes) for transposes/DVE."""
    t = a.tensor
    if t.dtype != R32:
        return a
    t2 = bass.SBTensorHandle(name=t.name, shape=t.shape, dtype=F32,
                             base_partition=t.base_partition,
                             manual_sbuf_range=t.manual_sbuf_range,
                             manual_base_name=t.manual_base_name)
    return bass.AP(tensor=t2, offset=a.offset,
                   ap=[list(d) for d in a.ap])


def rview(a):
    """Alias an fp32 AP as float32r (same bytes, PE fast-path dtype)."""
    t = a.tensor
    t2 = bass.SBTensorHandle(name=t.name, shape=t.shape, dtype=R32,
                             base_partition=t.base_partition,
                             manual_sbuf_range=t.manual_sbuf_range,
                             manual_base_name=t.manual_base_name)
    return bass.AP(tensor=t2, offset=a.offset,
                   ap=[list(d) for d in a.ap])


def dram_ap(handle, dims, off=0):
    """AP over DRAM handle with fully explicit dims (first = partition)."""
    a = handle[:]
    return bass.AP(tensor=a.tensor, offset=a.offset + off,
                   ap=[list(d) for d in dims])


def build_program(split_waits=True):
    nc = bass.Bass("TRN2", target_bir_lowering=False)

    x_d = nc.dram_tensor("x", [S, I, H], F32, kind="ExternalInput")
    cst_d = nc.dram_tensor("cst", [P, CSTN], F32, kind="ExternalInput")
    out_d = nc.dram_tensor("out", [S, OF], F32, kind="ExternalOutput")

    with tile.TileContext(nc) as tc:
        with (
            tc.tile_pool(name="consts", bufs=1) as consts,
            tc.tile_pool(name="xpool", bufs=4) as xpool,
            tc.tile_pool(name="xtpool", bufs=4) as xtpool,
            tc.tile_pool(name="work", bufs=2) as work,
            tc.tile_pool(name="sm", bufs=2) as sm,
            tc.tile_pool(name="ps_big", bufs=1, space="PSUM") as ps_big,
            tc.tile_pool(name="ps_mid", bufs=2, space="PSUM") as ps_mid,
            tc.tile_pool(name="ps_xt", bufs=2, space="PSUM") as ps_xt,
            tc.tile_pool(name="ps_anch", bufs=1, space="PSUM") as ps_anch,
        ):
            # ---- constants: ONE DMA so downstream PE ops wait on one sem ----
            cst = consts.tile([P, CSTN], F32)
            nc.sync.dma_start(out=cst[:], in_=cst_d[:])
            w_sb = cst[:, CW:CW + 2 * OF]        # [h%128, (hc, of)]
            wt_sb = cst[:, CWT:CWT + 4 * H]      # [of%128, (m, h)]
            id_sb = cst[:, CID:CID + P]          # identity
            mc_sb = cst[:, CMC:CMC + OF]         # diag mask (p%32 == o)
            s4_sb = cst[:, CS4:CS4 + PK]         # s4[p,s] = (p//32 == s)
            bm_sb = cst[:, CBM:CBM + 4 * O]      # bm[p,(m,j)]=(j==8m+p//16)
            c0_sb = cst[:, CC0:CC0 + O]          # uniform 1/32 (iter 0)

            # PE sync anchors: every datapath instruction carries at most ONE
            # sem wait (walrus).  A 1x1 transpose reading a byte of a dirty
            # foreign-engine tensor makes PE "observe" that engine's clock so
            # later PE instructions need no cross-engine waits.
            anch = ps_anch.tile([P, F], F32)
            dirty = {}
            acol = [0]
            pending = []

            def mark(key, apv):
                dirty[key] = apv

            def pe_sync(*keys):
                pending.clear()
                for k in keys:
                    if k not in dirty:
                        continue
                    d = dirty.pop(k)
                    a = nc.tensor.transpose(
                        anch[:1, acol[0]:acol[0] + 1], d[:1, :1],
                        id_sb[:1, :1])
                    pending.append(a.ins)
                    acol[0] = (acol[0] + 1) % F

            def _chain(b):
                for a in pending:
                    bass._add_dep_helper(b.ins, a, sync=False,
                                         reason="pe-anchor order")
                return b

            def T(out, in_, ident):
                return _chain(nc.tensor.transpose(out, in_, ident))

            def MM(out, lhsT, rhs, **kw):
                return _chain(nc.tensor.matmul(out, lhsT, rhs, **kw))

            def dep(b, a):
                if a is not None:
                    bass._add_dep_helper(b.ins, a, sync=False,
                                         reason="engine-anchor order")
                return b

            mark("cst", cst)

            dscr = sm.tile([PK, PK], F32, tag="dscr")
            nc.vector.memset(dscr[:], 0.0)
            # one-time: let DVE observe the const DMA (mc/bm reads)
            dcst_a = nc.vector.tensor_copy(dscr[:1, :1], cst[:1, :1]).ins

            # ---- load x (natural [i, h]); build xT via PE transposes ----
            x_sb = {}   # (hb, pk) -> flat [128, (s, ic, h)] = [128, 4096]
            xt_sb = {}  # (hb, pk) -> flat [128, (s, hc, i)] = [128, 4096]
            for hb in range(NHB):
                for pk in range(NPK):
                    samp0 = hb * 8 + pk * 4
                    xs = xpool.tile([P, PK * 4 * H], F32, tag="x")
                    nc.sync.dma_start(
                        out=ap(xs, [[4 * H, PK], [H, 4], [1, H]]),
                        in_=dram_ap(x_d, [[H, P], [I * H, PK], [P * H, 4], [1, H]],
                                    off=samp0 * I * H),
                    )
                    mark("x", xs)
                    x_sb[(hb, pk)] = xs
                    xt = xtpool.tile([P, PK * 2 * I], F32, tag="xt")
                    for s in range(PK):
                        for hc in range(2):
                            pe_sync("cst", "x", "act", "dve")
                            pxt = ps_xt.tile([P, I], F32, tag="pxt")
                            for ic in range(4):
                                T(
                                    pxt[:, ic * P:(ic + 1) * P],
                                    xs[:, s * 1024 + ic * H + hc * P:
                                           s * 1024 + ic * H + (hc + 1) * P],
                                    id_sb,
                                )
                            dst = xt[:, s * 1024 + hc * I: s * 1024 + (hc + 1) * I]
                            if pk == 0:
                                nc.scalar.activation(dst, pxt[:], AF.Copy)
                                mark("act", dst)
                            else:
                                nc.vector.tensor_copy(dst, pxt[:])
                                mark("dve", dst)
                    xt_sb[(hb, pk)] = xt

            cT = {0: None, 1: None}  # per-hb [128 i%128, (pk, ic, s, o)]
            for hb in range(NHB):
                for t in range(NITER):
                    # ---- y = c @ x : [(pk,) 4s*32o', 256h] ----
                    pe_sync("cst", "x", "dve", "act")
                    ps_y = ps_mid.tile([P, NPK * H], F32, tag="mid")
                    for pk in range(NPK):
                        for s in range(PK):
                            for ic in range(4):
                                lhsT = (c0_sb if t == 0 else
                                        ap(cT[hb], [[1, O]],
                                           off=pk * 512 + ic * P + s * O))
                                MM(
                                    ps_y[32 * s:32 * s + 32,
                                         pk * H:(pk + 1) * H],
                                    lhsT,
                                    ap(x_sb[hb, pk], [[1, H]],
                                       off=s * 1024 + ic * H),
                                    start=(ic == 0),
                                    stop=(ic == 3),
                                    tile_position=(0, 32 * s),
                                )
                    y_sb = work.tile([P, NPK * H], F32, tag="ysb")
                    nc.scalar.activation(y_sb[:], ps_y[:], AF.Copy)
                    mark("act", y_sb)

                    # ---- yT : [h, (pk, 4s*32o')] ----
                    pe_sync("act", "dve")
                    ps_yt = ps_mid.tile([P, NPK * 2 * P], F32, tag="mid")
                    for pk in range(NPK):
                        for hc in range(2):
                            T(
                                ps_yt[:, (pk * 2 + hc) * P:(pk * 2 + hc + 1) * P],
                                y_sb[:, pk * H + hc * P: pk * H + (hc + 1) * P],
                                id_sb,
                            )
                    yt_sb = work.tile([P, NPK * 2 * P], F32, tag="ytsb")
                    nc.vector.tensor_copy(yt_sb[:], ps_yt[:])
                    mark("dve", yt_sb)

                    # ---- vfull = y @ W : [(pk,) 4s*32o', (o,f)] ----
                    pe_sync("dve", "act")
                    ps_vf = ps_big.tile([P, NPK * OF], F32, tag="big")
                    for pk in range(NPK):
                        for hc in range(2):
                            MM(
                                ps_vf[:, pk * OF:(pk + 1) * OF],
                                yt_sb[:, (pk * 2 + hc) * P:(pk * 2 + hc + 1) * P],
                                ap(w_sb, [[1, OF]], off=hc * OF),
                                start=(hc == 0),
                                stop=(hc == 1),
                            )

                    # ---- diag extract: mask, then per-sample partition sum ----
                    msk_sb = work.tile([P, NPK * OF], F32, tag="bigsb")
                    dep(nc.vector.tensor_mul(
                        ap(msk_sb, [[OF, NPK], [1, OF]]),
                        ap(ps_vf, [[OF, NPK], [1, OF]]),
                        ap(mc_sb, [[0, NPK], [1, OF]]),
                    ), dcst_a)
                    mark("dve", msk_sb)
                    pe_sync("dve")
                    ps_vr = ps_big.tile([PK, NPK * OF], F32, tag="big")
                    for pk in range(NPK):
                        MM(
                            ps_vr[:, pk * OF:(pk + 1) * OF],
                            s4_sb,
                            msk_sb[:, pk * OF:(pk + 1) * OF],
                            start=True, stop=True,
                        )

                    # ---- squash: factor = sqrt(mag)/(1+mag), Newton step ----
                    vr_sb = work.tile([PK, NPK * OF], F32, tag="vrsb", bufs=1)
                    nc.scalar.activation(vr_sb[:], ps_vr[:], AF.Copy)
                    mark("act", vr_sb)
                    sq_sb = work.tile([PK, NPK * OF], F32, tag="sqsb", bufs=1)
                    nc.scalar.activation(sq_sb[:], vr_sb[:], AF.Square)
                    mag = sm.tile([PK, NPK * O], F32, tag="mag")
                    nc.vector.reduce_sum(
                        out=mag[:],
                        in_=ap(sq_sb, [[F, NPK * O], [1, F]]),
                        axis=AX.X,
                    )
                    s0 = sm.tile([PK, NPK * O], F32, tag="s0")
                    nc.scalar.activation(s0[:], mag[:], AF.Sqrt)
                    mark("act", s0)
                    r0 = sm.tile([PK, NPK * O], F32, tag="r0")
                    nc.vector.reciprocal(r0[:], s0[:])
                    t1 = sm.tile([PK, NPK * O], F32, tag="t1")
                    nc.vector.tensor_mul(t1[:], mag[:], r0[:])
                    sadd = sm.tile([PK, NPK * O], F32, tag="sadd")
                    nc.vector.tensor_add(sadd[:], s0[:], t1[:])   # ~2 sqrt(mag)
                    onep = sm.tile([PK, NPK * O], F32, tag="onep")
                    nc.vector.tensor_scalar_add(onep[:], mag[:], 1.0)
                    rec = sm.tile([PK, NPK * O], F32, tag="rec")
                    nc.vector.reciprocal(rec[:], onep[:])
                    rec2 = sm.tile([PK, NPK * O], F32, tag="rec2")
                    nc.vector.tensor_scalar_mul(rec2[:], rec[:], 0.5)
                    factor = sm.tile([PK, NPK * O], F32, tag="fac")
                    nc.vector.tensor_mul(factor[:], sadd[:], rec2[:])

                    vsq = work.tile([PK, NPK * OF], F32, tag="vsq", bufs=3)
                    nc.vector.tensor_mul(
                        ap(vsq, [[F, NPK * O], [1, F]]),
                        ap(vr_sb, [[F, NPK * O], [1, F]]),
                        ap(factor, [[1, NPK * O], [0, F]]),
                    )
                    mark("dve", vsq)

                    if t == NITER - 1:
                        nc.sync.dma_start(
                            out=dram_ap(out_d, [[OF, PK], [PK * OF, NPK], [1, OF]],
                                        off=hb * 8 * OF),
                            in_=ap(vsq, [[OF, NPK], [1, OF]]),
                        )
                        continue

                    # ---- vT chunks: [(o8,f16)%128, (pk, m, s)] ----
                    pe_sync("dve", "act")
                    ps_vt = ps_mid.tile([P, NPK * 4 * PK], F32, tag="mid")
                    for pk in range(NPK):
                        for m in range(4):
                            T(
                                ps_vt[:, (pk * 4 + m) * PK:(pk * 4 + m + 1) * PK],
                                vsq[:, pk * OF + m * P: pk * OF + (m + 1) * P],
                                id_sb[:PK, :PK],
                            )
                    vt_sb = work.tile([P, NPK * 4 * PK], F32, tag="vtsb")
                    nc.vector.tensor_copy(vt_sb[:], ps_vt[:])

                    # ---- Vmat blocks: vp[p,(pk,m,s,j)] = vt * blockmask ----
                    vp_sb = work.tile([P, NPK * 4 * PK * O], F32, tag="vp", bufs=1)
                    for m in range(4):
                        nc.vector.tensor_mul(
                            ap(vp_sb, [[4 * PK * O, NPK], [O, PK], [1, O]],
                               off=m * PK * O),
                            ap(vt_sb, [[4 * PK, NPK], [1, PK], [0, O]],
                               off=m * PK),
                            ap(bm_sb, [[0, NPK], [0, PK], [1, O]],
                               off=m * O),
                        )
                    mark("dve", vp_sb[:, 3 * PK * O: 3 * PK * O + 1])

                    # ---- z = Vmat @ WT : [(pk,) 4s*32o, 256h] ----
                    pe_sync("dve", "act")
                    ps_z = ps_mid.tile([P, NPK * H], F32, tag="mid")
                    for pk in range(NPK):
                        for s in range(PK):
                            for m in range(4):
                                MM(
                                    ps_z[32 * s:32 * s + 32,
                                         pk * H:(pk + 1) * H],
                                    ap(vp_sb, [[1, O]],
                                       off=pk * 512 + m * P + s * O),
                                    ap(wt_sb, [[1, H]], off=m * H),
                                    start=(m == 0),
                                    stop=(m == 3),
                                    tile_position=(0, 32 * s),
                                )
                    z_sb = work.tile([P, NPK * H], F32, tag="ysb")
                    nc.scalar.activation(z_sb[:], ps_z[:], AF.Copy)
                    mark("act", z_sb)

                    # ---- zT ----
                    pe_sync("act", "dve")
                    ps_zt = ps_mid.tile([P, NPK * 2 * P], F32, tag="mid")
                    for pk in range(NPK):
                        for hc in range(2):
                            T(
                                ps_zt[:, (pk * 2 + hc) * P:(pk * 2 + hc + 1) * P],
                                z_sb[:, pk * H + hc * P: pk * H + (hc + 1) * P],
                                id_sb,
                            )
                    zt_sb = work.tile([P, NPK * 2 * P], F32, tag="ytsb")
                    nc.vector.tensor_copy(zt_sb[:], ps_zt[:])
                    mark("dve", zt_sb)

                    # ---- b = z @ xT : [(pk,) 4s*32o, 512i] ----
                    pe_sync("dve", "act")
                    ps_b = ps_big.tile([P, NPK * I], F32, tag="big")
                    for pk in range(NPK):
                        for s in range(PK):
                            for hc in range(2):
                                MM(
                                    ps_b[32 * s:32 * s + 32,
                                         pk * I:(pk + 1) * I],
                                    ap(zt_sb, [[1, O]],
                                       off=(pk * 2 + hc) * P + 32 * s),
                                    ap(xt_sb[hb, pk], [[1, I]],
                                       off=s * 1024 + hc * I),
                                    start=(hc == 0),
                                    stop=(hc == 1),
                                    tile_position=(0, 32 * s),
                                )

                    # ---- softmax over o (b in +-5 => exp w/o max-subtract) ----
                    expb = work.tile([P, NPK * I], F32, tag="bigsb")
                    nc.scalar.activation(expb[:], ps_b[:], AF.Exp)
                    mark("act", expb)

                    pe_sync("act", "dve")
                    ps_ebt = ps_big.tile([P, NPK * 4 * P], F32, tag="big")
                    for pk in range(NPK):
                        for ic in range(4):
                            T(
                                ps_ebt[:, (pk * 4 + ic) * P:(pk * 4 + ic + 1) * P],
                                expb[:, pk * I + ic * P: pk * I + (ic + 1) * P],
                                id_sb,
                            )
                    ebt = work.tile([P, NPK * 4 * P], F32, tag="ebt", bufs=1)
                    nc.scalar.activation(ebt[:], ps_ebt[:], AF.Copy)
                    mark("act", ebt)

                    ssum = sm.tile([P, NPK * 4 * PK], F32, tag="ssum")
                    nc.vector.reduce_sum(
                        out=ssum[:],
                        in_=ap(ebt, [[O, NPK * 4 * PK], [1, O]]),
                        axis=AX.X,
                    )
                    rsum = sm.tile([P, NPK * 4 * PK], F32, tag="rsum")
                    nc.vector.reciprocal(rsum[:], ssum[:])
                    cT[hb] = work.tile([P, NPK * 4 * P], F32, tag="ct%d" % hb,
                                       bufs=1, name="ct_t")
                    nc.vector.tensor_mul(
                        ap(cT[hb], [[O, NPK * 4 * PK], [1, O]]),
                        ap(ebt, [[O, NPK * 4 * PK], [1, O]]),
                        ap(rsum, [[1, NPK * 4 * PK], [0, O]]),
                    )
                    mark("dve", cT[hb])

    if split_waits:
        _split_fat_waits(nc)
    return nc


def _split_fat_waits(nc, maxw=1):
    """Walrus caps sync waits per instruction; split overflow onto extra
    same-engine Drain instructions inserted just before the offender."""
    nsplit = 0
    for blk in nc.m.functions[0].blocks:
        new_insts = []
        for inst in blk.instructions:
            si = getattr(inst, "sync_info", None)
            w = list(si.on_wait) if si is not None and si.on_wait else []
            if len(w) > maxw:
                for k in range(0, len(w) - maxw, maxw):
                    d = mybir.InstDrain(name="I-waitsplit-%d" % nsplit,
                                        ins=[], outs=[])
                    nsplit += 1
                    d.engine = inst.engine
                    d.sync_info = mybir.SyncInfo(on_wait=w[k:k + maxw],
                                                 on_update=[])
                    new_insts.append(d)
                si.on_wait = w[len(w) - maxw:]
            new_insts.append(inst)
        blk.instructions[:] = new_insts
    return nc


_NC_CACHE = None


def make_cst(Wn):
    """Constant blob [128, CSTN] matching the device-side layout."""
    cst = np.zeros((P, CSTN), np.float32)
    # W [h, of] -> [h%128, (hc, of)]
    cst[:, CW:CW + 2 * OF] = Wn.reshape(2, P, OF).transpose(1, 0, 2).reshape(P, 2 * OF)
    # WT [of, h] -> [of%128, (m, h)]
    cst[:, CWT:CWT + 4 * H] = (
        Wn.T.reshape(4, P, H).transpose(1, 0, 2).reshape(P, 4 * H))
    cst[:, CID:CID + P] = np.eye(P, dtype=np.float32)
    for p in range(P):
        o = p % O
        cst[p, CMC + o * F:CMC + (o + 1) * F] = 1.0
    cst[np.arange(P), CS4 + np.arange(P) // O] = 1.0
    for p in range(P):
        for m in range(4):
            cst[p, CBM + m * O + 8 * m + p // F] = 1.0
    cst[:, CC0:CC0 + O] = 1.0 / O
    return cst


def make_in_maps(x, W):
    x = np.ascontiguousarray(np.asarray(x, dtype=np.float32))
    Wn = np.ascontiguousarray(np.asarray(W, dtype=np.float32).reshape(H, OF))
    cst = make_cst(Wn)
    xs = x.reshape(NCORES, S, I, H)
    return [
        {"x": np.ascontiguousarray(xs[c]), "cst": cst}
        for c in range(NCORES)
    ]


def kernel(x: np.ndarray, W: np.ndarray) -> np.ndarray:
    global _NC_CACHE
    if _NC_CACHE is None:
        _NC_CACHE = build_program()
    in_maps = make_in_maps(x, W)
    res = run_bass_kernel_spmd(_NC_CACHE, in_maps, core_ids=list(range(NCORES)))
    out = np.stack([res.results[c]["out"] for c in range(NCORES)])
    return out.reshape(B, O, F)



# revision 24
# speedup vs baseline: 1.0390x; 1.0390x over previous
"""Trainium2 Bass kernel for nn_Capsule (capsule routing with dynamic routing).

reference: u = x @ W  (per-sample [512,256]@[256,512] -> [512, (32 o, 16 f)])
           b=0; 3x { c = softmax_o(b); v[o,f] = sum_i c[o,i] u[i,(o,f)];
                     v = squash(v); b[o,i] = sum_f v[o,f] u[i,(o,f)] }
           return v  [B, 32, 16]

Key algebraic restructuring (u is NEVER materialized):
  v_raw[o,f] = sum_i c[o,i] u[i,(o,f)]  =  diag-extract[ (c @ x) @ W ]
      y = c @ x      (PE bf16: cT stationary [i,32], x natural moving)
      vfull = y @ W  (PE fp32r: yT stationary, W natural moving)
      v_raw = mask * vfull, then per-sample partition-sum via indicator matmul
  b[o,i] = sum_f v[o,f] u[i,(o,f)] = sum_h z[o,h] x[i,h]
      zT[h,(s,o)] = WT-chunk stationary @ VmatT moving (PE fp32r, dst-0,
                    256-wide moving: 4x fewer rows than the dense z form)
      b = z @ xT     (PE bf16: zT stationary col-tiled, xT moving)
  softmax over o on bT [i-partition, o-free] via PE transposes of exp(b).

Dtype split: fp32r (fp32 bits, PE fast path, requires dst partition 0) for
vfull / vr / flipped-z; bf16 for the column-tiled y and b phases (fp32r
forbids tile_position col offsets) plus x / xT / cT / zT / exp(b).
x is cast to bf16 on the HOST (halves HBM traffic for the dominant input).

16 samples/core x 8 cores; per core 2 half-batches of 2 packs x 4 samples;
a pack's 4 samples run concurrently via PE column tiling tile_position=(0,32s).
"""

import numpy as np
import ml_dtypes

import concourse.bass as bass
import concourse.tile as tile
from concourse import mybir
from concourse.bass_utils import run_bass_kernel_spmd

F32 = mybir.dt.float32
R32 = mybir.dt.float32r
BF16 = mybir.dt.bfloat16
AF = mybir.ActivationFunctionType
AX = mybir.AxisListType

B, I, H = 128, 512, 256
O, F = 32, 16
OF = O * F  # 512
NCORES = 8
S = B // NCORES      # 16 samples per core
NHB = 2              # half-batches per core
NPK = 2              # packs per half-batch
PK = 4               # samples per pack (col-tiling width)
NITER = 3
P = 128

# fp32 constant blob layout (one DMA, per-partition element offsets)
CW = 0                  # W  [h%128, (hc 2, of 512)]
CWT = CW + 2 * OF       # WT [of%128, (m 4, h 256)]
CID = CWT + 4 * H       # identity [128, 128]
CMC = CID + P           # diag mask [128, 512]
CS4 = CMC + OF          # sample-sum indicator [128, 4]
CBM = CS4 + PK          # Vmat block masks [128, (m 4, j 32)]
CSTN = CBM + 4 * O

# bf16 constant blob
CBID = 0                # identity [128, 128] bf16
CBC0 = CBID + P         # uniform 1/32 [128, 32] bf16
CBN = CBC0 + O


def ap(t, dims, off=0):
    """AP over tile/handle `t`: keep partition dim, explicit free dims."""
    a = t if isinstance(t, bass.AP) else t[:]
    return bass.AP(tensor=a.tensor, offset=a.offset + off,
                   ap=[list(a.ap[0])] + [list(d) for d in dims])


def fview(a):
    """Alias a float32r AP as plain fp32 (same bytes) for transposes/DVE."""
    t = a.tensor
    if t.dtype != R32:
        return a
    t2 = bass.SBTensorHandle(name=t.name, shape=t.shape, dtype=F32,
                             base_partition=t.base_partition,
                             manual_sbuf_range=t.manual_sbuf_range,
                             manual_base_name=t.manual_base_name)
    return bass.AP(tensor=t2, offset=a.offset,
                   ap=[list(d) for d in a.ap])


def dram_ap(handle, dims, off=0):
    """AP over DRAM handle with fully explicit dims (first = partition)."""
    a = handle[:]
    return bass.AP(tensor=a.tensor, offset=a.offset + off,
                   ap=[list(d) for d in dims])


def build_program(split_waits=True, loop_n=None):
    """loop_n: wrap the whole body in a hardware For_i loop (timing runs)."""
    import contextlib

    nc = bass.Bass("TRN2", target_bir_lowering=False)

    x_d = nc.dram_tensor("x", [S, I, H], BF16, kind="ExternalInput")
    cst_d = nc.dram_tensor("cst", [P, CSTN], R32, kind="ExternalInput")
    cstb_d = nc.dram_tensor("cstb", [P, CBN], BF16, kind="ExternalInput")
    out_d = nc.dram_tensor("out", [S, OF], F32, kind="ExternalOutput")

    with tile.TileContext(nc) as tc:
        with (
            tc.tile_pool(name="consts", bufs=1) as consts,
            tc.tile_pool(name="xpool", bufs=4) as xpool,
            tc.tile_pool(name="xtpool", bufs=4) as xtpool,
            tc.tile_pool(name="work", bufs=2) as work,
            tc.tile_pool(name="sm", bufs=2) as sm,
            tc.tile_pool(name="ps_big", bufs=1, space="PSUM") as ps_big,
            tc.tile_pool(name="ps_mid", bufs=2, space="PSUM") as ps_mid,
            tc.tile_pool(name="ps_xt", bufs=2, space="PSUM") as ps_xt,
            tc.tile_pool(name="ps_anch", bufs=1, space="PSUM") as ps_anch,
            tc.For_i(0, loop_n, 1) if loop_n else contextlib.nullcontext(),
        ):
            # ---- constants ----
            cstb = consts.tile([P, CBN], BF16)
            nc.sync.dma_start(out=cstb[:], in_=cstb_d[:])
            idb_sb = cstb[:, CBID:CBID + P]      # identity bf16
            c0b_sb = cstb[:, CBC0:CBC0 + O]      # uniform 1/32 bf16

            cst = consts.tile([P, CSTN], R32)
            nc.sync.dma_start(out=cst[:], in_=cst_d[:])
            w_sb = cst[:, CW:CW + 2 * OF]        # [h%128, (hc, of)] R32
            wt_sb = cst[:, CWT:CWT + 4 * H]      # [of%128, (m, h)] R32
            idr_sb = cst[:, CID:CID + P]         # identity (R32 transposes)
            id_sb = fview(idr_sb)                # identity (fp32 transposes)
            mc_sb = fview(cst[:, CMC:CMC + OF])  # diag mask (p%32 == o)
            s4_sb = cst[:, CS4:CS4 + PK]         # s4[p,s] = (p//32 == s) R32
            bm_sb = fview(cst[:, CBM:CBM + 4 * O])  # bm[p,(m,j)]=(j==8m+p//16)

            # PE sync anchors: every datapath instruction carries at most ONE
            # sem wait (walrus).  A 1x1 transpose reading a byte of a dirty
            # foreign-engine tensor makes PE "observe" that engine's clock so
            # later PE instructions need no cross-engine waits.
            anch = ps_anch.tile([P, F], F32)
            anchb = ps_anch.tile([P, 2 * F], BF16)  # even cols: 4B alignment
            dirty = {}
            acol = [0]
            pending = []

            def mark(key, apv):
                dirty[key] = apv

            def pe_sync(*keys):
                pending.clear()
                for k in keys:
                    if k not in dirty:
                        continue
                    d = dirty.pop(k)
                    dd = fview(d[:1, :1])
                    if dd.tensor.dtype == BF16:
                        a = nc.tensor.transpose(
                            anchb[:1, 2 * acol[0]:2 * acol[0] + 1], dd,
                            idb_sb[:1, :1])
                    else:
                        a = nc.tensor.transpose(
                            anch[:1, acol[0]:acol[0] + 1], dd,
                            id_sb[:1, :1])
                    pending.append(a.ins)
                    acol[0] = (acol[0] + 1) % F
            def _chain(b):
                for a in pending:
                    bass._add_dep_helper(b.ins, a, sync=False,
                                         reason="pe-anchor order")
                return b

            def T(out, in_, ident):
                return _chain(nc.tensor.transpose(out, in_, ident))

            def MM(out, lhsT, rhs, **kw):
                return _chain(nc.tensor.matmul(out, lhsT, rhs, **kw))

            def dep(b, a):
                if a is not None:
                    bass._add_dep_helper(b.ins, a, sync=False,
                                         reason="engine-anchor order")
                return b

            mark("cst", cst)
            mark("cstb", cstb)

            dscr = sm.tile([PK, PK], F32, tag="dscr")
            nc.vector.memset(dscr[:], 0.0)
            # one-time: let DVE observe the const DMA (mc/bm reads)
            dcst_a = nc.vector.tensor_copy(dscr[:1, :1],
                                           fview(cst[:1, :1])).ins

            # ---- load x (natural [i, h], bf16); build xT via PE transposes
            x_sb = {}   # (hb, pk) -> flat [128, (s, ic, h)] = [128, 4096]
            xt_sb = {}  # (hb, pk) -> flat [128, (s, hc, i)] = [128, 4096]
            for hb in range(NHB):
                for pk in range(NPK):
                    samp0 = hb * 8 + pk * 4
                    xs = xpool.tile([P, PK * 4 * H], BF16, tag="x")
                    nc.sync.dma_start(
                        out=ap(xs, [[4 * H, PK], [H, 4], [1, H]]),
                        in_=dram_ap(x_d, [[H, P], [I * H, PK], [P * H, 4], [1, H]],
                                    off=samp0 * I * H),
                    )
                    mark("x", xs)
                    x_sb[(hb, pk)] = xs
                    xt = xtpool.tile([P, PK * 2 * I], BF16, tag="xt")
                    for s in range(PK):
                        for hc in range(2):
                            pe_sync("cstb", "x", "act", "dve")
                            pxt = ps_xt.tile([P, I], BF16, tag="pxt")
                            for ic in range(4):
                                T(
                                    pxt[:, ic * P:(ic + 1) * P],
                                    xs[:, s * 1024 + ic * H + hc * P:
                                           s * 1024 + ic * H + (hc + 1) * P],
                                    idb_sb,
                                )
                            dst = xt[:, s * 1024 + hc * I: s * 1024 + (hc + 1) * I]
                            if pk == 0:
                                nc.scalar.activation(dst, pxt[:], AF.Copy)
                                mark("act", dst)
                            else:
                                nc.vector.tensor_copy(dst, pxt[:])
                                mark("dve", dst)
                    xt_sb[(hb, pk)] = xt

            cT = {0: None, 1: None}  # per-hb bf16 [128 i%128, (pk, ic, s, o)]
            for hb in range(NHB):
                for t in range(NITER):
                    # ---- y = c @ x (bf16): [(pk,) 4s*32o', 256h] ----
                    pe_sync("cstb", "x", "dve", "act")
                    ps_y = ps_mid.tile([P, NPK * H], F32, tag="mid")
                    for pk in range(NPK):
                        for s in range(PK):
                            for ic in range(4):
                                lhsT = (c0b_sb if t == 0 else
                                        ap(cT[hb], [[1, O]],
                                           off=pk * 512 + ic * P + s * O))
                                MM(
                                    ps_y[32 * s:32 * s + 32,
                                         pk * H:(pk + 1) * H],
                                    lhsT,
                                    ap(x_sb[hb, pk], [[1, H]],
                                       off=s * 1024 + ic * H),
                                    start=(ic == 0),
                                    stop=(ic == 3),
                                    tile_position=(0, 32 * s),
                                )
                    y_sb = work.tile([P, NPK * H], R32, tag="ysb")
                    nc.scalar.activation(y_sb[:], ps_y[:], AF.Copy)
                    mark("act", y_sb)

                    # ---- yT (fp32r): [h, (pk, 4s*32o')] ----
                    pe_sync("act", "dve")
                    ps_yt = ps_mid.tile([P, NPK * 2 * P], R32, tag="mid")
                    for pk in range(NPK):
                        for hc in range(2):
                            T(
                                ps_yt[:, (pk * 2 + hc) * P:(pk * 2 + hc + 1) * P],
                                y_sb[:, pk * H + hc * P: pk * H + (hc + 1) * P],
                                idr_sb,
                            )
                    yt_sb = work.tile([P, NPK * 2 * P], R32, tag="ytsb")
                    nc.vector.tensor_copy(yt_sb[:], ps_yt[:])
                    mark("dve", yt_sb)

                    # ---- vfull = y @ W (fp32r): [(pk,) 4s*32o', (o,f)] ----
                    pe_sync("dve", "act")
                    ps_vf = ps_big.tile([P, NPK * OF], F32, tag="big")
                    for pk in range(NPK):
                        for hc in range(2):
                            MM(
                                ps_vf[:, pk * OF:(pk + 1) * OF],
                                yt_sb[:, (pk * 2 + hc) * P:(pk * 2 + hc + 1) * P],
                                ap(w_sb, [[1, OF]], off=hc * OF),
                                start=(hc == 0),
                                stop=(hc == 1),
                            )

                    # ---- diag extract: mask, then per-sample partition sum ----
                    msk_sb = work.tile([P, NPK * OF], R32, tag="bigsb")
                    dep(nc.vector.tensor_mul(
                        ap(msk_sb, [[OF, NPK], [1, OF]]),
                        ap(ps_vf, [[OF, NPK], [1, OF]]),
                        ap(mc_sb, [[0, NPK], [1, OF]]),
                    ), dcst_a)
                    mark("dve", msk_sb)
                    pe_sync("dve")
                    ps_vr = ps_big.tile([PK, NPK * OF], F32, tag="big")
                    for pk in range(NPK):
                        MM(
                            ps_vr[:, pk * OF:(pk + 1) * OF],
                            s4_sb,
                            msk_sb[:, pk * OF:(pk + 1) * OF],
                            start=True, stop=True,
                        )

                    # ---- squash: factor = sqrt(mag)/(1+mag), Newton step ----
                    vr_sb = work.tile([PK, NPK * OF], F32, tag="vrsb", bufs=1)
                    nc.scalar.activation(vr_sb[:], ps_vr[:], AF.Copy)
                    mark("act", vr_sb)
                    sq_sb = work.tile([PK, NPK * OF], F32, tag="sqsb", bufs=1)
                    nc.scalar.activation(sq_sb[:], vr_sb[:], AF.Square)
                    mag = sm.tile([PK, NPK * O], F32, tag="mag")
                    nc.vector.reduce_sum(
                        out=mag[:],
                        in_=ap(sq_sb, [[F, NPK * O], [1, F]]),
                        axis=AX.X,
                    )
                    s0 = sm.tile([PK, NPK * O], F32, tag="s0")
                    nc.scalar.activation(s0[:], mag[:], AF.Sqrt)
                    mark("act", s0)
                    r0 = sm.tile([PK, NPK * O], F32, tag="r0")
                    nc.vector.reciprocal(r0[:], s0[:])
                    t1 = sm.tile([PK, NPK * O], F32, tag="t1")
                    nc.vector.tensor_mul(t1[:], mag[:], r0[:])
                    sadd = sm.tile([PK, NPK * O], F32, tag="sadd")
                    nc.vector.tensor_add(sadd[:], s0[:], t1[:])   # ~2 sqrt(mag)
                    onep = sm.tile([PK, NPK * O], F32, tag="onep")
                    nc.vector.tensor_scalar_add(onep[:], mag[:], 1.0)
                    rec = sm.tile([PK, NPK * O], F32, tag="rec")
                    nc.vector.reciprocal(rec[:], onep[:])
                    rec2 = sm.tile([PK, NPK * O], F32, tag="rec2")
                    nc.vector.tensor_scalar_mul(rec2[:], rec[:], 0.5)
                    factor = sm.tile([PK, NPK * O], F32, tag="fac")
                    nc.vector.tensor_mul(factor[:], sadd[:], rec2[:])

                    vsq = work.tile([PK, NPK * OF], F32, tag="vsq", bufs=3)
                    nc.vector.tensor_mul(
                        ap(vsq, [[F, NPK * O], [1, F]]),
                        ap(vr_sb, [[F, NPK * O], [1, F]]),
                        ap(factor, [[1, NPK * O], [0, F]]),
                    )
                    mark("dve", vsq)

                    if t == NITER - 1:
                        nc.sync.dma_start(
                            out=dram_ap(out_d, [[OF, PK], [PK * OF, NPK], [1, OF]],
                                        off=hb * 8 * OF),
                            in_=ap(vsq, [[OF, NPK], [1, OF]]),
                        )
                        continue

                    # ---- vT chunks: [(o8,f16)%128, (pk, m, s)] ----
                    pe_sync("dve", "act")
                    ps_vt = ps_mid.tile([P, NPK * 4 * PK], F32, tag="mid")
                    for pk in range(NPK):
                        for m in range(4):
                            T(
                                ps_vt[:, (pk * 4 + m) * PK:(pk * 4 + m + 1) * PK],
                                vsq[:, pk * OF + m * P: pk * OF + (m + 1) * P],
                                id_sb[:PK, :PK],
                            )
                    vt_sb = work.tile([P, NPK * 4 * PK], F32, tag="vtsb")
                    nc.vector.tensor_copy(vt_sb[:], ps_vt[:])

                    # ---- VmatT blocks: vp2[p,(m,pk,s,o)] = vtT * blockmask ----
                    vp2_sb = work.tile([P, 4 * NPK * PK * O], R32, tag="vp",
                                       bufs=1)
                    for m in range(4):
                        nc.vector.tensor_mul(
                            ap(vp2_sb, [[PK * O, NPK], [O, PK], [1, O]],
                               off=m * NPK * PK * O),
                            ap(vt_sb, [[4 * PK, NPK], [1, PK], [0, O]],
                               off=m * PK),
                            ap(bm_sb, [[0, NPK], [0, PK], [1, O]],
                               off=m * O),
                        )
                    mark("dve", vp2_sb[:, 3 * NPK * PK * O:
                                       3 * NPK * PK * O + 1])

                    # ---- zT = W(chunk)T-stationary @ VmatT (fp32r, dst 0):
                    #      [h%128, (hc, pk, s, o)] ----
                    pe_sync("dve", "act")
                    ps_zt2 = ps_mid.tile([P, NPK * H], F32, tag="mid")
                    for hc in range(2):
                        for m in range(4):
                            MM(
                                ps_zt2[:, hc * NPK * P:(hc + 1) * NPK * P],
                                ap(wt_sb, [[1, P]], off=m * H + hc * P),
                                ap(vp2_sb, [[1, NPK * P]],
                                   off=m * NPK * P),
                                start=(m == 0),
                                stop=(m == 3),
                            )
                    zt2_sb = work.tile([P, NPK * H], BF16, tag="ztsb")
                    nc.vector.tensor_copy(zt2_sb[:], ps_zt2[:])
                    mark("dve", zt2_sb)

                    # ---- b = z @ xT (bf16): [(pk,) 4s*32o, 512i] ----
                    pe_sync("dve", "act")
                    ps_b = ps_big.tile([P, NPK * I], F32, tag="big")
                    for pk in range(NPK):
                        for s in range(PK):
                            for hc in range(2):
                                MM(
                                    ps_b[32 * s:32 * s + 32,
                                         pk * I:(pk + 1) * I],
                                    ap(zt2_sb, [[1, O]],
                                       off=hc * NPK * P + pk * P + 32 * s),
                                    ap(xt_sb[hb, pk], [[1, I]],
                                       off=s * 1024 + hc * I),
                                    start=(hc == 0),
                                    stop=(hc == 1),
                                    tile_position=(0, 32 * s),
                                )

                    # ---- softmax over o (b in +-5 => exp w/o max-subtract) ----
                    expb = work.tile([P, NPK * I], BF16, tag="expb")
                    nc.scalar.activation(expb[:], ps_b[:], AF.Exp)
                    mark("act", expb)

                    pe_sync("act", "dve")
                    ps_ebt = ps_big.tile([P, NPK * 4 * P], BF16, tag="big")
                    for pk in range(NPK):
                        for ic in range(4):
                            T(
                                ps_ebt[:, (pk * 4 + ic) * P:(pk * 4 + ic + 1) * P],
                                expb[:, pk * I + ic * P: pk * I + (ic + 1) * P],
                                idb_sb,
                            )
                    ebt = work.tile([P, NPK * 4 * P], BF16, tag="ebt", bufs=1)
                    nc.scalar.activation(ebt[:], ps_ebt[:], AF.Copy)
                    mark("act", ebt)

                    ssum = sm.tile([P, NPK * 4 * PK], F32, tag="ssum")
                    nc.vector.reduce_sum(
                        out=ssum[:],
                        in_=ap(ebt, [[O, NPK * 4 * PK], [1, O]]),
                        axis=AX.X,
                    )
                    rsum = sm.tile([P, NPK * 4 * PK], F32, tag="rsum")
                    nc.vector.reciprocal(rsum[:], ssum[:])
                    cT[hb] = work.tile([P, NPK * 4 * P], BF16, tag="ct%d" % hb,
                                       bufs=1, name="ct_t")
                    nc.vector.tensor_mul(
                        ap(cT[hb], [[O, NPK * 4 * PK], [1, O]]),
                        ap(ebt, [[O, NPK * 4 * PK], [1, O]]),
                        ap(rsum, [[1, NPK * 4 * PK], [0, O]]),
                    )
                    mark("dve", cT[hb])

    if split_waits:
        _split_fat_waits(nc)
    return nc


def _split_fat_waits(nc, maxw=1):
    """Walrus caps sync waits per instruction; split overflow onto extra
    same-engine Drain instructions inserted just before the offender."""
    nsplit = 0
    for blk in nc.m.functions[0].blocks:
        new_insts = []
        for inst in blk.instructions:
            si = getattr(inst, "sync_info", None)
            w = list(si.on_wait) if si is not None and si.on_wait else []
            if len(w) > maxw:
                for k in range(0, len(w) - maxw, maxw):
                    d = mybir.InstDrain(name="I-waitsplit-%d" % nsplit,
                                        ins=[], outs=[])
                    nsplit += 1
                    d.engine = inst.engine
                    d.sync_info = mybir.SyncInfo(on_wait=w[k:k + maxw],
                                                 on_update=[])
                    new_insts.append(d)
                si.on_wait = w[len(w) - maxw:]
            new_insts.append(inst)
        blk.instructions[:] = new_insts
    return nc


_NC_CACHE = None


def make_cst(Wn):
    """fp32 constant blob [128, CSTN] matching the device-side layout."""
    cst = np.zeros((P, CSTN), np.float32)
    # W [h, of] -> [h%128, (hc, of)]
    cst[:, CW:CW + 2 * OF] = Wn.reshape(2, P, OF).transpose(1, 0, 2).reshape(P, 2 * OF)
    # WT [of, h] -> [of%128, (m, h)]
    cst[:, CWT:CWT + 4 * H] = (
        Wn.T.reshape(4, P, H).transpose(1, 0, 2).reshape(P, 4 * H))
    cst[:, CID:CID + P] = np.eye(P, dtype=np.float32)
    for p in range(P):
        o = p % O
        cst[p, CMC + o * F:CMC + (o + 1) * F] = 1.0
    cst[np.arange(P), CS4 + np.arange(P) // O] = 1.0
    for p in range(P):
        for m in range(4):
            cst[p, CBM + m * O + 8 * m + p // F] = 1.0
    return cst


def make_cstb():
    """bf16 constant blob [128, CBN]: identity + uniform 1/32."""
    cb = np.zeros((P, CBN), ml_dtypes.bfloat16)
    cb[:, CBID:CBID + P] = np.eye(P, dtype=ml_dtypes.bfloat16)
    cb[:, CBC0:CBC0 + O] = ml_dtypes.bfloat16(1.0 / O)
    return cb


def make_in_maps(x, W):
    x = np.asarray(x, dtype=np.float32)
    xb = np.ascontiguousarray(x.astype(ml_dtypes.bfloat16))
    Wn = np.ascontiguousarray(np.asarray(W, dtype=np.float32).reshape(H, OF))
    cst = make_cst(Wn)
    cstb = make_cstb()
    xs = xb.reshape(NCORES, S, I, H)
    return [
        {"x": np.ascontiguousarray(xs[c]), "cst": cst, "cstb": cstb}
        for c in range(NCORES)
    ]


def kernel(x: np.ndarray, W: np.ndarray) -> np.ndarray:
    global _NC_CACHE
    if _NC_CACHE is None:
        _NC_CACHE = build_program()
    in_maps = make_in_maps(x, W)
    res = run_bass_kernel_spmd(_NC_CACHE, in_maps, core_ids=list(range(NCORES)))
    out = np.stack([res.results[c]["out"] for c in range(NCORES)])
    return out.reshape(B, O, F)
